# revision 26
# baseline (speedup 1.0000x reference)
"""MoE layer (GShard top-2 routing + per-expert FFN) on 8 Trainium2 NeuronCores.

Strategy (expert parallelism, ReduceScatter return path):
  - Router matmul (fp32, exact) is token-sharded: each core computes logits for
    its 1024-token shard, then an AllGather shares per-token routing scalars
    (idx1, idx2, g1, g2) with all cores.
  - Every core replicates the (cheap) global slot-assignment math: per-expert
    inclusive scans along the free dim + a triangular-matmul partition prefix
    give each token its capacity slot exactly as the reference's cumsum does.
  - Each core owns ONE expert. The slot->gid map (gid = choice*T + token) is
    built with local_scatter (per-partition scatter by slot), merged across
    partitions with a ones-matmul, and read out column-major via a diagonal
    extraction. tokc = gid mod T gives the dispatch/scatter row; per-slot
    gates come from 16 indirect row gathers of g12[gid] (off critical path).
  - Dispatch: 16 indirect row gathers from x (bf16) + PE transposes give the
    [d, slot] layout; FFN in bf16 with fp32 accumulation:
    hT = gelu(w_gate^T @ dispT), eo = g_slot * (hT^T @ w_down) with the gate
    multiply folded into the PSUM->SBUF copy.
  - Return: each block's gated eo rows are indirect-scattered to rs_in[token]
    (empty/dropped slots fall on row T and are dropped by bounds_check); a
    single ReduceScatter(add) over [T, D] bf16 sums the two expert
    contributions per token and leaves shard m's rows on core m = y directly.
  - Weight loads and the rs_in zero-fill are chunked into ~1MB DMAs so the
    small router-payload DMA is never stuck behind a 23us transfer.
"""

import sys

if "/opt/trn_rl_repo" not in sys.path:
    sys.path.insert(0, "/opt/trn_rl_repo")

import numpy as np
import ml_dtypes

import concourse.bacc as bacc
import concourse.mybir as mybir
import concourse.tile as tile
from concourse import bass
from concourse.bass_utils import run_bass_kernel_spmd

BF16 = mybir.dt.bfloat16
F32 = mybir.dt.float32
I16 = mybir.dt.int16
I32 = mybir.dt.int32
AF = mybir.ActivationFunctionType
OP = mybir.AluOpType

B, S, D, E, F = 4, 2048, 1024, 8, 4096
T = B * S            # 8192 tokens
C = 2 * T // E       # 2048 capacity
NC = 8               # cores
SH = T // NC         # 1024 tokens per shard
CBLK = 512           # FFN slot-block
NCB = C // CBLK      # 4 blocks

LAST_RESULT = None   # BassKernelResults of the most recent run (for profiling)


def _build_program():
    nc = bacc.Bacc("TRN2", target_bir_lowering=False, debug=False, num_devices=NC)

    # ---- per-core external inputs ----
    xT_sh = nc.dram_tensor("xT_sh", [D, SH], F32, kind="ExternalInput").ap()
    xb = nc.dram_tensor("xb", [T + 1, D], BF16, kind="ExternalInput").ap()
    wg_d = nc.dram_tensor("wg", [D, E], F32, kind="ExternalInput").ap()
    wgt_d = nc.dram_tensor("wgt", [D, F], BF16, kind="ExternalInput").ap()
    wdn_d = nc.dram_tensor("wdn", [F, D], BF16, kind="ExternalInput").ap()
    cid_d = nc.dram_tensor("cid", [128, 1], F32, kind="ExternalInput").ap()
    # host-generated constants (gpsimd iota/affine_select aren't available)
    ident_d = nc.dram_tensor("ident", [128, 128], F32, kind="ExternalInput").ap()
    slmat_d = nc.dram_tensor("slmat", [128, 128], F32, kind="ExternalInput").ap()
    tidx_d = nc.dram_tensor("tidx", [128, 64], F32, kind="ExternalInput").ap()
    eidx_d = nc.dram_tensor("eidx", [128, E], F32, kind="ExternalInput").ap()
    y_d = nc.dram_tensor("y", [SH, D], BF16, kind="ExternalOutput").ap()

    # ---- internal DRAM ----
    pay_in = nc.dram_tensor("pay_in", [4 * SH], F32).ap()
    pay_all = nc.dram_tensor("pay_all", [NC * 4 * SH], F32, addr_space="Shared").ap()
    g12_d = nc.dram_tensor("g12", [2 * T + 128, 1], F32).ap()
    rs_in = nc.dram_tensor("rs_in", [T, D], BF16).ap()
    rs_out = nc.dram_tensor("rs_out", [SH, D], BF16).ap()

    with tile.TileContext(nc) as tc:
        with (
            tc.tile_pool(name="persist", bufs=1) as pp,
            tc.tile_pool(name="psum_s", bufs=2, space="PSUM") as pss,
        ):
            cid = pp.tile([128, 1], F32)
            zeros64 = pp.tile([128, 64], F32)
            nc.vector.memset(zeros64[:], 0.0)
            ones128 = pp.tile([128, 128], F32)
            nc.vector.memset(ones128[:], 1.0)
            # zero-fill source for rs_in; memset deliberately deferred until
            # block-0 dispatch is done (DVE order) so the 32 zero-fill DMAs
            # can't crowd the DMA engines ahead of the critical path gathers
            zt = pp.tile([128, 2, D], BF16)

            # resident gate weight (bf16); DMAs issued after the router
            # section (chunked so small DMAs can interleave); wdn_sb lives in
            # the FFN pool (not needed until mm2) to relieve SBUF pressure
            wgt_sb = pp.tile([128, D // 128, F], BF16)

            # persistent routing products
            tokci = pp.tile([128, C // 128], I32)   # dispatch+return row (t or T)
            gidi = pp.tile([128, C // 128], I32)    # gid = c*T + t (2T if empty)
            gsl = pp.tile([128, C // 128], F32)     # per-slot gate (keep folded)

            # =============== ROUTER (token shard, fp32) ===============
            with tc.tile_pool(name="route", bufs=1) as pr:
                xT_sb = pr.tile([128, D // 128, SH], F32)
                nc.sync.dma_start(xT_sb[:], xT_sh.rearrange("(o q) t -> q o t", q=128))
                wg_sb = pr.tile([128, D // 128, E], F32)
                nc.sync.dma_start(wg_sb[:], wg_d.rearrange("(o q) e -> q o e", q=128))
                ident = pr.tile([128, 128], F32)
                nc.sync.dma_start(ident[:], ident_d[:])
                nc.sync.dma_start(cid[:], cid_d[:])

                lg = pr.tile([128, 8, E], F32)  # logits, token pos j = 128*tt + p
                for tt in range(8):
                    ps = pss.tile([128, E], F32, space="PSUM", tag="ps_small")
                    for kd in range(8):
                        nc.tensor.matmul(
                            ps[:],
                            lhsT=xT_sb[:, kd, 128 * tt : 128 * tt + 128],
                            rhs=wg_sb[:, kd, :],
                            start=(kd == 0),
                            stop=(kd == 7),
                        )
                    nc.vector.tensor_copy(lg[:, tt, :], ps[:])

                def emax(src, width, tag):
                    cur = src
                    w = width
                    while w > 1:
                        nxt = pr.tile([128, 8, w // 2], F32, tag=f"emax{tag}{w}")
                        nc.vector.tensor_tensor(
                            out=nxt[:], in0=cur[:, :, : w // 2], in1=cur[:, :, w // 2 :],
                            op=OP.max,
                        )
                        cur, w = nxt, w // 2
                    return cur  # [128, 8, 1]

                m1x = emax(lg, E, "m1")
                is1 = pr.tile([128, 8, E], F32)
                nc.vector.tensor_tensor(out=is1[:], in0=lg[:], in1=m1x[:].to_broadcast([128, 8, E]), op=OP.is_equal)
                l2 = pr.tile([128, 8, E], F32)
                nc.vector.scalar_tensor_tensor(
                    out=l2[:], in0=is1[:], scalar=-1e30, in1=lg[:], op0=OP.mult, op1=OP.add,
                )
                m2x = emax(l2, E, "m2")
                is2 = pr.tile([128, 8, E], F32)
                nc.vector.tensor_tensor(out=is2[:], in0=l2[:], in1=m2x[:].to_broadcast([128, 8, E]), op=OP.is_equal)

                dm = pr.tile([128, 8, 1], F32)
                nc.vector.tensor_tensor(out=dm[:], in0=m2x[:], in1=m1x[:], op=OP.subtract)
                e2 = pr.tile([128, 8, 1], F32)
                nc.scalar.activation(e2[:], dm[:], AF.Exp)
                den = pr.tile([128, 8, 1], F32)
                nc.vector.tensor_scalar_add(den[:], e2[:], 1.0)
                g1 = pr.tile([128, 8, 1], F32)
                nc.vector.reciprocal(g1[:], den[:])
                g2 = pr.tile([128, 8, 1], F32)
                nc.vector.tensor_tensor(out=g2[:], in0=e2[:], in1=g1[:], op=OP.mult)

                eidx = pr.tile([128, E], F32)
                nc.sync.dma_start(eidx[:], eidx_d[:])

                def argmax_num(mask, tag):
                    t1 = pr.tile([128, 8, E], F32, tag=f"am_t1{tag}")
                    nc.vector.tensor_tensor(
                        out=t1[:], in0=mask[:], in1=eidx[:, None, :].to_broadcast([128, 8, E]), op=OP.mult,
                    )
                    cur, w = t1, E
                    while w > 1:
                        nxt = pr.tile([128, 8, w // 2], F32, tag=f"am_s{tag}{w}")
                        nc.vector.tensor_tensor(
                            out=nxt[:], in0=cur[:, :, : w // 2], in1=cur[:, :, w // 2 :], op=OP.add,
                        )
                        cur, w = nxt, w // 2
                    return cur  # [128, 8, 1]

                i1f = argmax_num(is1, "a")
                i2f = argmax_num(is2, "b")

                pk = pr.tile([128, 4, 8], F32)
                nc.vector.tensor_copy(pk[:, 0, :], i1f[:, :, 0])
                nc.vector.tensor_copy(pk[:, 1, :], i2f[:, :, 0])
                nc.vector.tensor_copy(pk[:, 2, :], g1[:, :, 0])
                nc.vector.tensor_copy(pk[:, 3, :], g2[:, :, 0])
                nc.sync.dma_start(pay_in.rearrange("(a p tt) -> p a tt", a=4, p=128), pk[:])

                nc.gpsimd.collective_compute(
                    "AllGather", OP.bypass,
                    replica_groups=[list(range(NC))],
                    ins=[pay_in[:].opt()], outs=[pay_all[:].opt()],
                )

                # reread all 4 arrays into global routing layout [128, 64] (t = 64p + i)
                rt = pr.tile([128, 4, 64], F32)
                pay_view = pay_all.rearrange("(r a p16 i) -> r p16 a i", r=NC, a=4, p16=16)
                for r in range(NC):
                    nc.sync.dma_start(rt[16 * r : 16 * r + 16, :, :], pay_view[r])
                i1r, i2r = rt[:, 0, :], rt[:, 1, :]
                g1r, g2r = rt[:, 2, :], rt[:, 3, :]

                # =============== SLOT ASSIGNMENT (replicated) ===============
                m1 = pr.tile([128, E, 64], F32)
                m2 = pr.tile([128, E, 64], F32)
                sc1 = pr.tile([128, E, 64], F32)
                sc2 = pr.tile([128, E, 64], F32)
                eidx_b = eidx[:, :, None].to_broadcast([128, E, 64])
                nc.vector.tensor_tensor(out=m1[:], in0=i1r[:, None, :].to_broadcast([128, E, 64]), in1=eidx_b, op=OP.is_equal)
                nc.vector.tensor_tensor(out=m2[:], in0=i2r[:, None, :].to_broadcast([128, E, 64]), in1=eidx_b, op=OP.is_equal)
                for e in range(E):
                    nc.vector.tensor_tensor_scan(sc1[:, e, :], m1[:, e, :], zeros64[:], 0.0, op0=OP.add, op1=OP.add)
                    nc.vector.tensor_tensor_scan(sc2[:, e, :], m2[:, e, :], zeros64[:], 0.0, op0=OP.add, op1=OP.add)
                tot1 = pr.tile([128, E], F32)
                tot2 = pr.tile([128, E], F32)
                nc.vector.tensor_copy(tot1[:], sc1[:, :, 63])
                nc.vector.tensor_copy(tot2[:], sc2[:, :, 63])

                sl = pr.tile([128, 128], F32)
                nc.sync.dma_start(sl[:], slmat_d[:])

                of1_ps = pss.tile([128, E], F32, space="PSUM", tag="ps_small")
                nc.tensor.matmul(of1_ps[:], lhsT=sl[:], rhs=tot1[:], start=True, stop=True)
                of1 = pr.tile([128, E], F32)
                nc.vector.tensor_scalar_add(of1[:], of1_ps[:], -1.0)
                of2_ps = pss.tile([128, E], F32, space="PSUM", tag="ps_small")
                nc.tensor.matmul(of2_ps[:], lhsT=sl[:], rhs=tot2[:], start=True, stop=False)
                nc.tensor.matmul(of2_ps[:], lhsT=ones128[:], rhs=tot1[:], start=False, stop=True)
                of2 = pr.tile([128, E], F32)
                nc.vector.tensor_scalar_add(of2[:], of2_ps[:], -1.0)

                def loc_s(sc, m, of, tag):
                    tmp = pr.tile([128, E, 64], F32, tag=f"loc_tmp{tag}")
                    for e in range(E):
                        nc.vector.scalar_tensor_tensor(
                            out=tmp[:, e, :], in0=sc[:, e, :], scalar=of[:, e : e + 1],
                            in1=m[:, e, :], op0=OP.add, op1=OP.mult,
                        )
                    cur, w = tmp, E
                    while w > 1:
                        nxt = pr.tile([128, w // 2, 64], F32, tag=f"loc_s{tag}{w}")
                        nc.vector.tensor_tensor(out=nxt[:], in0=cur[:, : w // 2, :], in1=cur[:, w // 2 :, :], op=OP.add)
                        cur, w = nxt, w // 2
                    return cur  # [128, 1, 64]

                l1s = loc_s(sc1, m1, of1, "a")[:, 0, :]
                l2s = loc_s(sc2, m2, of2, "b")[:, 0, :]

                def keep_gate(ls, gr, tag):
                    kp = pr.tile([128, 64], F32, tag=f"kp{tag}")
                    nc.vector.tensor_scalar(out=kp[:], in0=ls, scalar1=float(C), scalar2=None, op0=OP.is_lt)
                    gk = pr.tile([128, 64], F32, tag=f"gk{tag}")
                    nc.vector.tensor_tensor(out=gk[:], in0=gr, in1=kp[:], op=OP.mult)
                    return gk, kp

                g1k, kp1 = keep_gate(l1s, g1r, "a")
                g2k, kp2 = keep_gate(l2s, g2r, "b")

                # write per-token gates (keep folded) to DRAM for per-slot gathers:
                # g12[t] = g1k[t], g12[T + t] = g2k[t], g12[2T..] = 0 (empty slots)
                nc.sync.dma_start(
                    g12_d[0:T, :].rearrange("(p i) one -> p (i one)", p=128), g1k[:]
                )
                nc.sync.dma_start(
                    g12_d[T : 2 * T, :].rearrange("(p i) one -> p (i one)", p=128), g2k[:]
                )
                nc.sync.dma_start(
                    g12_d[2 * T : 2 * T + 128, :].rearrange("(p) one -> p (one)"),
                    zeros64[:, 0:1],
                )

                # ====== SLOT -> GID MAP (local_scatter + merge + diagonal) ======
                tif = pr.tile([128, 64], F32)
                nc.sync.dma_start(tif[:], tidx_d[:])
                tp1 = pr.tile([128, 64], F32)
                nc.vector.tensor_scalar_add(tp1[:], tif[:], 1.0)            # t + 1
                tp1b = pr.tile([128, 64], F32)
                nc.vector.tensor_scalar_add(tp1b[:], tif[:], float(T + 1))  # T + t + 1

                def slot_halves(ls, ir, kp, tag):
                    # sel = (expert == cid) && kept; slot+1 where selected else 0
                    isc = pr.tile([128, 64], F32, tag=f"isc{tag}")
                    nc.vector.tensor_tensor(out=isc[:], in0=ir, in1=cid[:, 0:1].to_broadcast([128, 64]), op=OP.is_equal)
                    sel = pr.tile([128, 64], F32, tag=f"sel{tag}")
                    nc.vector.tensor_tensor(out=sel[:], in0=isc[:], in1=kp[:], op=OP.mult)
                    sp1 = pr.tile([128, 64], F32, tag=f"sp1{tag}")  # sel ? slot+1 : 0
                    nc.vector.tensor_scalar_add(sp1[:], ls, 1.0)
                    nc.vector.tensor_tensor(out=sp1[:], in0=sp1[:], in1=sel[:], op=OP.mult)
                    # lo half: slot in [0, 1024): idx = slot, else -1
                    mlo = pr.tile([128, 64], F32, tag=f"mlo{tag}")
                    nc.vector.tensor_scalar(out=mlo[:], in0=sp1[:], scalar1=1024.0, scalar2=None, op0=OP.is_le)
                    nc.vector.tensor_tensor(out=mlo[:], in0=mlo[:], in1=sel[:], op=OP.mult)
                    ilo = pr.tile([128, 64], F32, tag=f"ilo{tag}")
                    nc.vector.tensor_tensor(out=ilo[:], in0=mlo[:], in1=sp1[:], op=OP.mult)
                    nc.vector.tensor_scalar_add(ilo[:], ilo[:], -1.0)
                    # hi half: slot in [1024, 2048): idx = slot - 1024, else -1
                    mhi = pr.tile([128, 64], F32, tag=f"mhi{tag}")
                    nc.vector.tensor_scalar(out=mhi[:], in0=sp1[:], scalar1=1024.0, scalar2=None, op0=OP.is_gt)
                    ihi = pr.tile([128, 64], F32, tag=f"ihi{tag}")
                    nc.vector.tensor_scalar_add(ihi[:], sp1[:], -1024.0)
                    nc.vector.tensor_tensor(out=ihi[:], in0=ihi[:], in1=mhi[:], op=OP.mult)
                    nc.vector.tensor_scalar_add(ihi[:], ihi[:], -1.0)
                    return ilo, ihi

                i1lo, i1hi = slot_halves(l1s, i1r, kp1, "a")
                i2lo, i2hi = slot_halves(l2s, i2r, kp2, "b")

                data128 = pr.tile([128, 128], I16)
                nc.vector.tensor_copy(data128[:, :64], tp1[:])
                nc.vector.tensor_copy(data128[:, 64:], tp1b[:])
                idxlo = pr.tile([128, 128], I16)
                nc.vector.tensor_copy(idxlo[:, :64], i1lo[:])
                nc.vector.tensor_copy(idxlo[:, 64:], i2lo[:])
                idxhi = pr.tile([128, 128], I16)
                nc.vector.tensor_copy(idxhi[:, :64], i1hi[:])
                nc.vector.tensor_copy(idxhi[:, 64:], i2hi[:])

                dst_lo = pr.tile([128, 1024], I16)
                nc.gpsimd.local_scatter(dst_lo[:], data128[:], idxlo[:], channels=128, num_elems=1024, num_idxs=128)
                dst_hi = pr.tile([128, 1024], I16)
                nc.gpsimd.local_scatter(dst_hi[:], data128[:], idxhi[:], channels=128, num_elems=1024, num_idxs=128)

                merged = pr.tile([128, 4, 512], F32)  # gid+1 replicated on all partitions
                for half, dst in ((0, dst_lo), (1, dst_hi)):
                    dstf = pr.tile([128, 1024], F32, tag="dstf")
                    nc.vector.tensor_copy(dstf[:], dst[:])
                    for ch in range(2):
                        mg_ps = pss.tile([128, 512], F32, space="PSUM", tag="ps_small")
                        nc.tensor.matmul(mg_ps[:], lhsT=ones128[:], rhs=dstf[:, 512 * ch : 512 * (ch + 1)], start=True, stop=True)
                        nc.vector.tensor_copy(merged[:, 2 * half + ch, :], mg_ps[:])

                # diagonal extraction: gidraw[p, k] = merged-flat[128k + p]
                gidraw = pr.tile([128, C // 128], F32)
                scratch = pr.tile([128, 128], F32, tag="diag_scr")
                mview = merged[:].rearrange("p a b -> p (a b)")
                for k in range(C // 128):
                    nc.vector.scalar_tensor_tensor(
                        out=scratch[:], in0=mview[:, 128 * k : 128 * (k + 1)], scalar=0.0,
                        in1=ident[:], op0=OP.add, op1=OP.mult,
                        accum_out=gidraw[:, k : k + 1],
                    )
                # sanitize: 0 -> 2T (empty slot); v -> v-1  => gid in [0,2T]
                iszero = pr.tile([128, C // 128], F32)
                nc.vector.tensor_scalar(out=iszero[:], in0=gidraw[:], scalar1=0.0, scalar2=None, op0=OP.is_equal)
                nc.vector.scalar_tensor_tensor(
                    out=gidraw[:], in0=iszero[:], scalar=float(2 * T + 1), in1=gidraw[:], op0=OP.mult, op1=OP.add,
                )
                nc.vector.tensor_scalar_add(gidraw[:], gidraw[:], -1.0)
                nc.vector.tensor_copy(gidi[:], gidraw[:])
                # tokf = gid - T*(gid >= T): token row in [0,T) or T (trash)
                cge = pr.tile([128, C // 128], F32)
                nc.vector.tensor_scalar(out=cge[:], in0=gidraw[:], scalar1=float(T), scalar2=None, op0=OP.is_ge)
                tokf = pr.tile([128, C // 128], F32)
                nc.vector.scalar_tensor_tensor(
                    out=tokf[:], in0=cge[:], scalar=float(-T), in1=gidraw[:], op0=OP.mult, op1=OP.add,
                )
                nc.vector.tensor_copy(tokci[:], tokf[:])

            # weight loads, chunked 1MB so small DMAs interleave
            for c in range(8):
                nc.sync.dma_start(
                    wgt_sb[:, :, 512 * c : 512 * (c + 1)],
                    wgt_d[:, 512 * c : 512 * (c + 1)].rearrange("(o q) f -> q o f", q=128),
                )
            # =============== EXPERT FFN (bf16) ===============
            with (
                tc.tile_pool(name="ffn", bufs=1) as pf,
                tc.tile_pool(name="ffn_db", bufs=2) as pfd,
                tc.tile_pool(name="ffn_drow", bufs=4) as pfg,
                tc.tile_pool(name="psum_mm", bufs=2, space="PSUM") as psm,
            ):
                wdn_sb = pf.tile([128, F // 128, D], BF16)
                # 128-row strided view of rs_in for the scatters: cost-model
                # sized to what is actually written, yet overlapping every
                # zero-fill chunk so Tile orders zeros -> scatters -> RS.
                scat_view = rs_in.rearrange("(a b) d -> b a d", b=64)[0]
                eo_tiles = {}
                dispT_tiles = {}

                def dispatch(cb):
                    # gather 4 x 128 slot rows; XBAR DMA transpose into dispT
                    # (keeps dispatch off the in-order PE queue entirely):
                    # dispT[p, j, c] = drow[c, 128j + p]
                    dispT = pfd.tile([128, D // 128, CBLK], BF16, tag="dispT")
                    dispT_tiles[cb] = dispT
                    for kt in range(CBLK // 128):
                        k = (CBLK // 128) * cb + kt
                        drow = pfg.tile([128, D], BF16, tag="drow")
                        nc.gpsimd.indirect_dma_start(
                            out=drow[:], out_offset=None, in_=xb[:],
                            in_offset=bass.IndirectOffsetOnAxis(ap=tokci[:, k : k + 1], axis=0),
                        )
                        nc.sync.dma_start_transpose(
                            dispT[:, :, 128 * kt : 128 * (kt + 1)], drow[:]
                        )
                    return drow

                dispatch(0)
                drow_gate = dispatch(1)
                # bulk loads gated behind block-1's last gather via a REAL
                # data dep (gate cells computed from drow_gate): Tile
                # schedules by dependency, not program order, so only a true
                # RAW edge keeps these DMAs out of the DMA queue until the
                # critical-path dispatch is done.
                gate_b = drow_gate[:, 0:1].to_broadcast([128, 2, D])
                for c in range(8):
                    nc.vector.tensor_scalar(
                        out=wdn_sb[:, 4 * c : 4 * c + 1, 0:1],
                        in0=drow_gate[:, 0:1], scalar1=0.0, scalar2=None, op0=OP.mult,
                    )
                    nc.sync.dma_start(
                        wdn_sb[:, 4 * c : 4 * (c + 1), :],
                        wdn_d[512 * c : 512 * (c + 1), :].rearrange("(o q) d -> q o d", q=128),
                    )
                nc.vector.tensor_scalar(
                    out=zt[:], in0=gate_b, scalar1=0.0, scalar2=None, op0=OP.mult,
                )
                for c in range(32):
                    nc.sync.dma_start(
                        rs_in[256 * c : 256 * (c + 1), :].rearrange("(q p) d -> p q d", p=128),
                        zt[:],
                    )
                # per-slot gates: gsl[p, k] = g12[gid]; garbage at empty
                # slots is harmless (their rows land on trash row T); created
                # last so their semaphore reuse never blocks dispatch
                for k in range(C // 128):
                    nc.gpsimd.indirect_dma_start(
                        out=gsl[:, k : k + 1], out_offset=None, in_=g12_d[:],
                        in_offset=bass.IndirectOffsetOnAxis(ap=gidi[:, k : k + 1], axis=0),
                    )

                for cb in range(NCB):
                    if cb >= 2:
                        dispatch(cb)
                    dispT = dispT_tiles.pop(cb)
                    if cb > 0:
                        # return previous block's gated rows to rs_in[token]
                        eo_prev = eo_tiles.pop(cb - 1)
                        for ct in range(CBLK // 128):
                            kprev = (CBLK // 128) * (cb - 1) + ct
                            nc.gpsimd.indirect_dma_start(
                                out=scat_view, in_=eo_prev[:, ct, :], in_offset=None,
                                out_offset=bass.IndirectOffsetOnAxis(ap=tokci[:, kprev : kprev + 1], axis=0),
                                bounds_check=T - 1, oob_is_err=False,
                            )

                    hT = pf.tile([128, F // 128, CBLK], BF16, tag="hT")
                    for ft in range(F // 128):
                        ps1 = psm.tile([128, CBLK], F32, space="PSUM", tag="ps1")
                        for kd in range(D // 128):
                            nc.tensor.matmul(
                                ps1[:],
                                lhsT=wgt_sb[:, kd, 128 * ft : 128 * ft + 128],
                                rhs=dispT[:, kd, :],
                                start=(kd == 0), stop=(kd == D // 128 - 1),
                            )
                        nc.scalar.activation(hT[:, ft, :], ps1[:], AF.Gelu)
                    # mm2 with swapped operands: eo[c, d] = g * (hT.T @ w_down)
                    eo_sb = pfd.tile([128, CBLK // 128, D], BF16, tag="eo_sb")
                    eo_tiles[cb] = eo_sb
                    for ct in range(CBLK // 128):
                        kcur = (CBLK // 128) * cb + ct
                        for dc in range(D // 512):
                            ps2 = psm.tile([128, 512], F32, space="PSUM", tag="ps2")
                            for ft in range(F // 128):
                                nc.tensor.matmul(
                                    ps2[:],
                                    lhsT=hT[:, ft, 128 * ct : 128 * ct + 128],
                                    rhs=wdn_sb[:, ft, 512 * dc : 512 * dc + 512],
                                    start=(ft == 0), stop=(ft == F // 128 - 1),
                                )
                            nc.vector.tensor_scalar_mul(
                                eo_sb[:, ct, 512 * dc : 512 * dc + 512], ps2[:],
                                gsl[:, kcur : kcur + 1],
                            )

                # last block's return scatters
                eo_prev = eo_tiles.pop(NCB - 1)
                for ct in range(CBLK // 128):
                    kprev = (CBLK // 128) * (NCB - 1) + ct
                    nc.gpsimd.indirect_dma_start(
                        out=scat_view, in_=eo_prev[:, ct, :], in_offset=None,
                        out_offset=bass.IndirectOffsetOnAxis(ap=tokci[:, kprev : kprev + 1], axis=0),
                        bounds_check=T - 1, oob_is_err=False,
                    )

                # sum the two expert contributions per token; shard m -> core m
                nc.gpsimd.collective_compute(
                    "ReduceScatter", OP.add,
                    replica_groups=[list(range(NC))],
                    ins=[rs_in[:].opt()], outs=[rs_out[:].opt()],
                )
                nc.sync.dma_start(y_d[:], rs_out[:])

    nc.compile()
    return nc


_PROGRAM = None


def _get_program():
    global _PROGRAM
    if _PROGRAM is None:
        _PROGRAM = _build_program()
    return _PROGRAM


def host_constants():
    p = np.arange(128)
    return {
        "ident": np.eye(128, dtype=np.float32),
        "slmat": (np.arange(128)[None, :] > p[:, None]).astype(np.float32),
        "tidx": (64 * p[:, None] + np.arange(64)[None, :]).astype(np.float32),
        "eidx": np.tile(np.arange(E, dtype=np.float32), (128, 1)),
    }


def _make_in_maps(x, wg, w_gate, w_down):
    x = np.asarray(x, np.float32)
    wg_np = np.asarray(wg, np.float32)
    w_gate_np = np.asarray(w_gate, np.float32)
    w_down_np = np.asarray(w_down, np.float32)

    tokens = x.reshape(T, D)
    xb = np.zeros((T + 1, D), ml_dtypes.bfloat16)
    xb[:T] = tokens.astype(ml_dtypes.bfloat16)

    # shard m holds tokens [SH*m, SH*(m+1)); its xT columns are permuted so that
    # matmul tile position j = 128*tt + p corresponds to local token 8*p + tt,
    # making the routing payload DMA contiguous.
    j = np.arange(SH)
    perm = 8 * (j % 128) + j // 128  # local token index at column position j
    consts = host_constants()

    in_maps = []
    for m in range(NC):
        shard = tokens[SH * m : SH * (m + 1)]
        xT_sh = np.ascontiguousarray(shard[perm].T)
        in_maps.append({
            "xT_sh": xT_sh,
            "xb": xb,
            "wg": wg_np,
            "wgt": np.ascontiguousarray(w_gate_np[m].astype(ml_dtypes.bfloat16)),
            "wdn": np.ascontiguousarray(w_down_np[m].astype(ml_dtypes.bfloat16)),
            "cid": np.full((128, 1), float(m), np.float32),
            **consts,
        })
    return in_maps


def kernel(x, wg, w_gate, w_down, _trace=False):
    global LAST_RESULT
    x = np.asarray(x, np.float32)
    in_maps = _make_in_maps(x, wg, w_gate, w_down)
    nc = _get_program()
    res = run_bass_kernel_spmd(nc, in_maps, core_ids=list(range(NC)), trace=_trace)
    LAST_RESULT = res
    out = np.concatenate([res.results[m]["y"] for m in range(NC)], axis=0)
    return out.reshape(B, S, D).astype(x.dtype)


def bench(x, wg, w_gate, w_down, iters=6):
    """Measure per-execution wall time with device-resident inputs.

    Returns (output, per_call_seconds_list).
    """
    import time
    import jax
    from jax.sharding import Mesh, PartitionSpec, NamedSharding
    from jax.experimental.shard_map import shard_map
    import concourse.mybir as _mybir
    from concourse.bass2jax import _bass_exec_p, install_neuronx_cc_hook, partition_id_tensor

    install_neuronx_cc_hook()
    nc = _get_program()

    x = np.asarray(x, np.float32)
    in_maps = _make_in_maps(x, wg, w_gate, w_down)

    in_names, out_names, out_avals, zero_outs = [], [], [], []
    for alloc in nc.m.functions[0].allocations:
        if not isinstance(alloc, _mybir.MemoryLocationSet):
            continue
        name = alloc.memorylocations[0].name
        if alloc.kind == "ExternalInput":
            if nc.partition_id_tensor is None or name != nc.partition_id_tensor.name:
                in_names.append(name)
        elif alloc.kind == "ExternalOutput":
            shape = tuple(alloc.tensor_shape)
            dtype = _mybir.dt.np(alloc.dtype)
            out_names.append(name)
            out_avals.append(jax.core.ShapedArray(shape, dtype))
            zero_outs.append(np.zeros(shape, dtype))
    n_params = len(in_names)
    all_in_names = in_names + out_names
    if nc.partition_id_tensor is not None:
        all_in_names = all_in_names + [nc.partition_id_tensor.name]

    def _body(*args):
        operands = list(args)
        if nc.partition_id_tensor is not None:
            operands.append(partition_id_tensor())
        outs = _bass_exec_p.bind(
            *operands,
            out_avals=tuple(out_avals),
            in_names=tuple(all_in_names),
            out_names=tuple(out_names),
            lowering_input_output_aliases=(),
            sim_require_finite=True,
            sim_require_nnan=True,
            nc=nc,
        )
        return tuple(outs)

    devices = jax.devices()[:NC]
    mesh = Mesh(np.asarray(devices), ("core",))
    nsh = NamedSharding(mesh, PartitionSpec("core"))
    n_outs = len(out_avals)
    donate = tuple(range(n_params, n_params + n_outs))
    sharded = jax.jit(
        shard_map(_body, mesh=mesh, in_specs=(PartitionSpec("core"),) * (n_params + n_outs),
                  out_specs=(PartitionSpec("core"),) * n_outs, check_rep=False),
        donate_argnums=donate, keep_unused=True,
    )

    concat_in = [
        jax.device_put(np.concatenate([np.asarray(in_maps[c][nm]) for c in range(NC)], axis=0), nsh)
        for nm in in_names
    ]
    zero_sets = [
        [jax.device_put(np.zeros((NC * z.shape[0], *z.shape[1:]), z.dtype), nsh) for z in zero_outs]
        for _ in range(iters + 1)
    ]

    out = sharded(*concat_in, *zero_sets[0])  # warmup + compile
    jax.block_until_ready(out)
    times = []
    for it in range(iters):
        t0 = time.perf_counter()
        out = sharded(*concat_in, *zero_sets[it + 1])
        jax.block_until_ready(out)
        times.append(time.perf_counter() - t0)

    outs = {
        nm: np.asarray(out[i]).reshape(NC, *out_avals[i].shape) for i, nm in enumerate(out_names)
    }
    y = np.concatenate([outs["y"][m] for m in range(NC)], axis=0).reshape(B, S, D).astype(x.dtype)
    return y, times


# revision 28
# speedup vs baseline: 1.0146x; 1.0146x over previous
"""MoE layer (GShard top-2 routing + per-expert FFN) on 8 Trainium2 NeuronCores.

Strategy (expert parallelism, ReduceScatter return path):
  - Router matmul (fp32, exact) is token-sharded: each core computes logits for
    its 1024-token shard, then an AllGather shares per-token routing scalars
    (idx1, idx2, g1, g2) with all cores.
  - Every core replicates the (cheap) global slot-assignment math: per-expert
    inclusive scans along the free dim + a triangular-matmul partition prefix
    give each token its capacity slot exactly as the reference's cumsum does.
  - Each core owns ONE expert. The slot->gid map (gid = choice*T + token) is
    built with local_scatter (per-partition scatter by slot), merged across
    partitions with a ones-matmul, and read out column-major via a diagonal
    extraction. tokc = gid mod T gives the dispatch/scatter row; per-slot
    gates come from 16 indirect row gathers of g12[gid] (off critical path).
  - Dispatch: 16 indirect row gathers from x (bf16) + PE transposes give the
    [d, slot] layout; FFN in bf16 with fp32 accumulation:
    hT = gelu(w_gate^T @ dispT), eo = g_slot * (hT^T @ w_down) with the gate
    multiply folded into the PSUM->SBUF copy.
  - Return: each block's gated eo rows are indirect-scattered to rs_in[token]
    (empty/dropped slots fall on row T and are dropped by bounds_check); a
    single ReduceScatter(add) over [T, D] bf16 sums the two expert
    contributions per token and leaves shard m's rows on core m = y directly.
  - Weight loads and the rs_in zero-fill are chunked into ~1MB DMAs so the
    small router-payload DMA is never stuck behind a 23us transfer.
"""

import sys

if "/opt/trn_rl_repo" not in sys.path:
    sys.path.insert(0, "/opt/trn_rl_repo")

import numpy as np
import ml_dtypes

import concourse.bacc as bacc
import concourse.mybir as mybir
import concourse.tile as tile
from concourse import bass
from concourse.bass_utils import run_bass_kernel_spmd

BF16 = mybir.dt.bfloat16
F32 = mybir.dt.float32
I16 = mybir.dt.int16
I32 = mybir.dt.int32
AF = mybir.ActivationFunctionType
OP = mybir.AluOpType

B, S, D, E, F = 4, 2048, 1024, 8, 4096
T = B * S            # 8192 tokens
C = 2 * T // E       # 2048 capacity
NC = 8               # cores
SH = T // NC         # 1024 tokens per shard
CBLK = 512           # FFN slot-block
NCB = C // CBLK      # 4 blocks

LAST_RESULT = None   # BassKernelResults of the most recent run (for profiling)


def _build_program():
    nc = bacc.Bacc("TRN2", target_bir_lowering=False, debug=False, num_devices=NC)

    # ---- per-core external inputs ----
    xT_sh = nc.dram_tensor("xT_sh", [D, SH], F32, kind="ExternalInput").ap()
    xb = nc.dram_tensor("xb", [T + 1, D], BF16, kind="ExternalInput").ap()
    wg_d = nc.dram_tensor("wg", [D, E], F32, kind="ExternalInput").ap()
    wgt_d = nc.dram_tensor("wgt", [D, F], BF16, kind="ExternalInput").ap()
    wdn_d = nc.dram_tensor("wdn", [F, D], BF16, kind="ExternalInput").ap()
    cid_d = nc.dram_tensor("cid", [128, 1], F32, kind="ExternalInput").ap()
    # host-generated constants (gpsimd iota/affine_select aren't available)
    ident_d = nc.dram_tensor("ident", [128, 128], F32, kind="ExternalInput").ap()
    slmat_d = nc.dram_tensor("slmat", [128, 128], F32, kind="ExternalInput").ap()
    tidx_d = nc.dram_tensor("tidx", [128, 64], F32, kind="ExternalInput").ap()
    eidx_d = nc.dram_tensor("eidx", [128, E], F32, kind="ExternalInput").ap()
    y_d = nc.dram_tensor("y", [SH, D], BF16, kind="ExternalOutput").ap()

    # ---- internal DRAM ----
    pay_in = nc.dram_tensor("pay_in", [4 * SH], F32).ap()
    pay_all = nc.dram_tensor("pay_all", [NC * 4 * SH], F32, addr_space="Shared").ap()
    rs_in = nc.dram_tensor("rs_in", [T, D], BF16).ap()
    rs_out = nc.dram_tensor("rs_out", [SH, D], BF16).ap()

    with tile.TileContext(nc) as tc:
        with (
            tc.tile_pool(name="persist", bufs=1) as pp,
            tc.tile_pool(name="psum_s", bufs=2, space="PSUM") as pss,
        ):
            cid = pp.tile([128, 1], F32)
            zeros64 = pp.tile([128, 64], F32)
            nc.vector.memset(zeros64[:], 0.0)
            ones128 = pp.tile([128, 128], F32)
            nc.vector.memset(ones128[:], 1.0)
            # zero-fill source for rs_in; memset deliberately deferred until
            # block-0 dispatch is done (DVE order) so the 32 zero-fill DMAs
            # can't crowd the DMA engines ahead of the critical path gathers
            zt = pp.tile([128, 2, D], BF16)

            # resident gate weight (bf16); DMAs issued after the router
            # section (chunked so small DMAs can interleave); wdn_sb lives in
            # the FFN pool (not needed until mm2) to relieve SBUF pressure
            wgt_sb = pp.tile([128, D // 128, F], BF16)

            # persistent routing products
            tokci = pp.tile([128, C // 128], I32)   # dispatch+return row (t or T)
            gsl = pp.tile([128, C // 128], F32)     # per-slot gate (keep folded)

            # =============== ROUTER (token shard, fp32) ===============
            with tc.tile_pool(name="route", bufs=1) as pr:
                xT_sb = pr.tile([128, D // 128, SH], F32)
                nc.sync.dma_start(xT_sb[:], xT_sh.rearrange("(o q) t -> q o t", q=128))
                wg_sb = pr.tile([128, D // 128, E], F32)
                nc.sync.dma_start(wg_sb[:], wg_d.rearrange("(o q) e -> q o e", q=128))
                ident = pr.tile([128, 128], F32)
                nc.sync.dma_start(ident[:], ident_d[:])
                nc.sync.dma_start(cid[:], cid_d[:])

                lg = pr.tile([128, 8, E], F32)  # logits, token pos j = 128*tt + p
                for tt in range(8):
                    ps = pss.tile([128, E], F32, space="PSUM", tag="ps_small")
                    for kd in range(8):
                        nc.tensor.matmul(
                            ps[:],
                            lhsT=xT_sb[:, kd, 128 * tt : 128 * tt + 128],
                            rhs=wg_sb[:, kd, :],
                            start=(kd == 0),
                            stop=(kd == 7),
                        )
                    nc.vector.tensor_copy(lg[:, tt, :], ps[:])

                def emax(src, width, tag):
                    cur = src
                    w = width
                    while w > 1:
                        nxt = pr.tile([128, 8, w // 2], F32, tag=f"emax{tag}{w}")
                        nc.vector.tensor_tensor(
                            out=nxt[:], in0=cur[:, :, : w // 2], in1=cur[:, :, w // 2 :],
                            op=OP.max,
                        )
                        cur, w = nxt, w // 2
                    return cur  # [128, 8, 1]

                m1x = emax(lg, E, "m1")
                is1 = pr.tile([128, 8, E], F32)
                nc.vector.tensor_tensor(out=is1[:], in0=lg[:], in1=m1x[:].to_broadcast([128, 8, E]), op=OP.is_equal)
                l2 = pr.tile([128, 8, E], F32)
                nc.vector.scalar_tensor_tensor(
                    out=l2[:], in0=is1[:], scalar=-1e30, in1=lg[:], op0=OP.mult, op1=OP.add,
                )
                m2x = emax(l2, E, "m2")
                is2 = pr.tile([128, 8, E], F32)
                nc.vector.tensor_tensor(out=is2[:], in0=l2[:], in1=m2x[:].to_broadcast([128, 8, E]), op=OP.is_equal)

                dm = pr.tile([128, 8, 1], F32)
                nc.vector.tensor_tensor(out=dm[:], in0=m2x[:], in1=m1x[:], op=OP.subtract)
                e2 = pr.tile([128, 8, 1], F32)
                nc.scalar.activation(e2[:], dm[:], AF.Exp)
                den = pr.tile([128, 8, 1], F32)
                nc.vector.tensor_scalar_add(den[:], e2[:], 1.0)
                g1 = pr.tile([128, 8, 1], F32)
                nc.vector.reciprocal(g1[:], den[:])
                g2 = pr.tile([128, 8, 1], F32)
                nc.vector.tensor_tensor(out=g2[:], in0=e2[:], in1=g1[:], op=OP.mult)

                eidx = pr.tile([128, E], F32)
                nc.sync.dma_start(eidx[:], eidx_d[:])

                def argmax_num(mask, tag):
                    t1 = pr.tile([128, 8, E], F32, tag=f"am_t1{tag}")
                    nc.vector.tensor_tensor(
                        out=t1[:], in0=mask[:], in1=eidx[:, None, :].to_broadcast([128, 8, E]), op=OP.mult,
                    )
                    cur, w = t1, E
                    while w > 1:
                        nxt = pr.tile([128, 8, w // 2], F32, tag=f"am_s{tag}{w}")
                        nc.vector.tensor_tensor(
                            out=nxt[:], in0=cur[:, :, : w // 2], in1=cur[:, :, w // 2 :], op=OP.add,
                        )
                        cur, w = nxt, w // 2
                    return cur  # [128, 8, 1]

                i1f = argmax_num(is1, "a")
                i2f = argmax_num(is2, "b")

                pk = pr.tile([128, 4, 8], F32)
                nc.vector.tensor_copy(pk[:, 0, :], i1f[:, :, 0])
                nc.vector.tensor_copy(pk[:, 1, :], i2f[:, :, 0])
                nc.vector.tensor_copy(pk[:, 2, :], g1[:, :, 0])
                nc.vector.tensor_copy(pk[:, 3, :], g2[:, :, 0])
                nc.sync.dma_start(pay_in.rearrange("(a p tt) -> p a tt", a=4, p=128), pk[:])

                nc.gpsimd.collective_compute(
                    "AllGather", OP.bypass,
                    replica_groups=[list(range(NC))],
                    ins=[pay_in[:].opt()], outs=[pay_all[:].opt()],
                )

                # reread all 4 arrays into global routing layout [128, 64] (t = 64p + i)
                rt = pr.tile([128, 4, 64], F32)
                pay_view = pay_all.rearrange("(r a p16 i) -> r p16 a i", r=NC, a=4, p16=16)
                for r in range(NC):
                    nc.sync.dma_start(rt[16 * r : 16 * r + 16, :, :], pay_view[r])
                i1r, i2r = rt[:, 0, :], rt[:, 1, :]
                g1r, g2r = rt[:, 2, :], rt[:, 3, :]

                # =============== SLOT ASSIGNMENT (replicated) ===============
                m1 = pr.tile([128, E, 64], F32)
                m2 = pr.tile([128, E, 64], F32)
                sc1 = pr.tile([128, E, 64], F32)
                sc2 = pr.tile([128, E, 64], F32)
                eidx_b = eidx[:, :, None].to_broadcast([128, E, 64])
                nc.vector.tensor_tensor(out=m1[:], in0=i1r[:, None, :].to_broadcast([128, E, 64]), in1=eidx_b, op=OP.is_equal)
                nc.vector.tensor_tensor(out=m2[:], in0=i2r[:, None, :].to_broadcast([128, E, 64]), in1=eidx_b, op=OP.is_equal)
                for e in range(E):
                    nc.vector.tensor_tensor_scan(sc1[:, e, :], m1[:, e, :], zeros64[:], 0.0, op0=OP.add, op1=OP.add)
                    nc.vector.tensor_tensor_scan(sc2[:, e, :], m2[:, e, :], zeros64[:], 0.0, op0=OP.add, op1=OP.add)
                tot1 = pr.tile([128, E], F32)
                tot2 = pr.tile([128, E], F32)
                nc.vector.tensor_copy(tot1[:], sc1[:, :, 63])
                nc.vector.tensor_copy(tot2[:], sc2[:, :, 63])

                sl = pr.tile([128, 128], F32)
                nc.sync.dma_start(sl[:], slmat_d[:])

                of1_ps = pss.tile([128, E], F32, space="PSUM", tag="ps_small")
                nc.tensor.matmul(of1_ps[:], lhsT=sl[:], rhs=tot1[:], start=True, stop=True)
                of1 = pr.tile([128, E], F32)
                nc.vector.tensor_scalar_add(of1[:], of1_ps[:], -1.0)
                of2_ps = pss.tile([128, E], F32, space="PSUM", tag="ps_small")
                nc.tensor.matmul(of2_ps[:], lhsT=sl[:], rhs=tot2[:], start=True, stop=False)
                nc.tensor.matmul(of2_ps[:], lhsT=ones128[:], rhs=tot1[:], start=False, stop=True)
                of2 = pr.tile([128, E], F32)
                nc.vector.tensor_scalar_add(of2[:], of2_ps[:], -1.0)

                def loc_s(sc, m, of, tag):
                    tmp = pr.tile([128, E, 64], F32, tag=f"loc_tmp{tag}")
                    for e in range(E):
                        nc.vector.scalar_tensor_tensor(
                            out=tmp[:, e, :], in0=sc[:, e, :], scalar=of[:, e : e + 1],
                            in1=m[:, e, :], op0=OP.add, op1=OP.mult,
                        )
                    cur, w = tmp, E
                    while w > 1:
                        nxt = pr.tile([128, w // 2, 64], F32, tag=f"loc_s{tag}{w}")
                        nc.vector.tensor_tensor(out=nxt[:], in0=cur[:, : w // 2, :], in1=cur[:, w // 2 :, :], op=OP.add)
                        cur, w = nxt, w // 2
                    return cur  # [128, 1, 64]

                l1s = loc_s(sc1, m1, of1, "a")[:, 0, :]
                l2s = loc_s(sc2, m2, of2, "b")[:, 0, :]

                def keep_gate(ls, gr, tag):
                    kp = pr.tile([128, 64], F32, tag=f"kp{tag}")
                    nc.vector.tensor_scalar(out=kp[:], in0=ls, scalar1=float(C), scalar2=None, op0=OP.is_lt)
                    gk = pr.tile([128, 64], F32, tag=f"gk{tag}")
                    nc.vector.tensor_tensor(out=gk[:], in0=gr, in1=kp[:], op=OP.mult)
                    return gk, kp

                g1k, kp1 = keep_gate(l1s, g1r, "a")
                g2k, kp2 = keep_gate(l2s, g2r, "b")

                # ====== SLOT -> GID MAP (local_scatter + merge + diagonal) ======
                tif = pr.tile([128, 64], F32)
                nc.sync.dma_start(tif[:], tidx_d[:])
                tp1 = pr.tile([128, 64], F32)
                nc.vector.tensor_scalar_add(tp1[:], tif[:], 1.0)            # t + 1

                def slot_halves(ls, ir, kp, tag):
                    # sel = (expert == cid) && kept; slot+1 where selected else 0
                    isc = pr.tile([128, 64], F32, tag=f"isc{tag}")
                    nc.vector.tensor_tensor(out=isc[:], in0=ir, in1=cid[:, 0:1].to_broadcast([128, 64]), op=OP.is_equal)
                    sel = pr.tile([128, 64], F32, tag=f"sel{tag}")
                    nc.vector.tensor_tensor(out=sel[:], in0=isc[:], in1=kp[:], op=OP.mult)
                    sp1 = pr.tile([128, 64], F32, tag=f"sp1{tag}")  # sel ? slot+1 : 0
                    nc.vector.tensor_scalar_add(sp1[:], ls, 1.0)
                    nc.vector.tensor_tensor(out=sp1[:], in0=sp1[:], in1=sel[:], op=OP.mult)
                    # lo half: slot in [0, 1024): idx = slot, else -1
                    mlo = pr.tile([128, 64], F32, tag=f"mlo{tag}")
                    nc.vector.tensor_scalar(out=mlo[:], in0=sp1[:], scalar1=1024.0, scalar2=None, op0=OP.is_le)
                    nc.vector.tensor_tensor(out=mlo[:], in0=mlo[:], in1=sel[:], op=OP.mult)
                    ilo = pr.tile([128, 64], F32, tag=f"ilo{tag}")
                    nc.vector.tensor_tensor(out=ilo[:], in0=mlo[:], in1=sp1[:], op=OP.mult)
                    nc.vector.tensor_scalar_add(ilo[:], ilo[:], -1.0)
                    # hi half: slot in [1024, 2048): idx = slot - 1024, else -1
                    mhi = pr.tile([128, 64], F32, tag=f"mhi{tag}")
                    nc.vector.tensor_scalar(out=mhi[:], in0=sp1[:], scalar1=1024.0, scalar2=None, op0=OP.is_gt)
                    ihi = pr.tile([128, 64], F32, tag=f"ihi{tag}")
                    nc.vector.tensor_scalar_add(ihi[:], sp1[:], -1024.0)
                    nc.vector.tensor_tensor(out=ihi[:], in0=ihi[:], in1=mhi[:], op=OP.mult)
                    nc.vector.tensor_scalar_add(ihi[:], ihi[:], -1.0)
                    return ilo, ihi

                i1lo, i1hi = slot_halves(l1s, i1r, kp1, "a")
                i2lo, i2hi = slot_halves(l2s, i2r, kp2, "b")

                data128 = pr.tile([128, 128], I16)
                nc.vector.tensor_copy(data128[:, :64], tp1[:])
                nc.vector.tensor_copy(data128[:, 64:], tp1[:])
                gdata = pr.tile([128, 128], BF16)   # per-token gates (keep folded)
                nc.vector.tensor_copy(gdata[:, :64], g1k[:])
                nc.vector.tensor_copy(gdata[:, 64:], g2k[:])
                idxlo = pr.tile([128, 128], I16)
                nc.vector.tensor_copy(idxlo[:, :64], i1lo[:])
                nc.vector.tensor_copy(idxlo[:, 64:], i2lo[:])
                idxhi = pr.tile([128, 128], I16)
                nc.vector.tensor_copy(idxhi[:, :64], i1hi[:])
                nc.vector.tensor_copy(idxhi[:, 64:], i2hi[:])

                dst_lo = pr.tile([128, 1024], I16)
                nc.gpsimd.local_scatter(dst_lo[:], data128[:], idxlo[:], channels=128, num_elems=1024, num_idxs=128)
                dst_hi = pr.tile([128, 1024], I16)
                nc.gpsimd.local_scatter(dst_hi[:], data128[:], idxhi[:], channels=128, num_elems=1024, num_idxs=128)
                gdst_lo = pr.tile([128, 1024], BF16)
                nc.gpsimd.local_scatter(gdst_lo[:], gdata[:], idxlo[:], channels=128, num_elems=1024, num_idxs=128)
                gdst_hi = pr.tile([128, 1024], BF16)
                nc.gpsimd.local_scatter(gdst_hi[:], gdata[:], idxhi[:], channels=128, num_elems=1024, num_idxs=128)
                ones_bf = pr.tile([128, 128], BF16)
                nc.vector.memset(ones_bf[:], 1.0)

                merged = pr.tile([128, 4, 512], F32)  # gid+1 replicated on all partitions
                for half, dst in ((0, dst_lo), (1, dst_hi)):
                    dstf = pr.tile([128, 1024], F32, tag="dstf")
                    nc.vector.tensor_copy(dstf[:], dst[:])
                    for ch in range(2):
                        mg_ps = pss.tile([128, 512], F32, space="PSUM", tag="ps_small")
                        nc.tensor.matmul(mg_ps[:], lhsT=ones128[:], rhs=dstf[:, 512 * ch : 512 * (ch + 1)], start=True, stop=True)
                        nc.vector.tensor_copy(merged[:, 2 * half + ch, :], mg_ps[:])

                # diagonal extraction: tokraw[p, k] = merged-flat[128k + p]
                tokraw = pr.tile([128, C // 128], F32)
                scratch = pr.tile([128, 128], F32, tag="diag_scr")
                mview = merged[:].rearrange("p a b -> p (a b)")
                for k in range(C // 128):
                    nc.vector.scalar_tensor_tensor(
                        out=scratch[:], in0=mview[:, 128 * k : 128 * (k + 1)], scalar=0.0,
                        in1=ident[:], op0=OP.add, op1=OP.mult,
                        accum_out=tokraw[:, k : k + 1],
                    )
                # sanitize: 0 -> T+1 (empty slot -> trash row); v -> v-1
                iszero = pr.tile([128, C // 128], F32)
                nc.vector.tensor_scalar(out=iszero[:], in0=tokraw[:], scalar1=0.0, scalar2=None, op0=OP.is_equal)
                nc.vector.scalar_tensor_tensor(
                    out=tokraw[:], in0=iszero[:], scalar=float(T + 1), in1=tokraw[:], op0=OP.mult, op1=OP.add,
                )
                nc.vector.tensor_scalar_add(tokraw[:], tokraw[:], -1.0)
                nc.vector.tensor_copy(tokci[:], tokraw[:])

                # per-slot gates, same scatter/merge/diagonal path (bf16 data;
                # empty slots read 0). Merge mms are bf16 so they are cheap.
                gmergd = pr.tile([128, 4, 512], F32)
                for half, gdst in ((0, gdst_lo), (1, gdst_hi)):
                    for ch in range(2):
                        gm_ps = pss.tile([128, 512], F32, space="PSUM", tag="ps_small")
                        nc.tensor.matmul(gm_ps[:], lhsT=ones_bf[:], rhs=gdst[:, 512 * ch : 512 * (ch + 1)], start=True, stop=True)
                        nc.vector.tensor_copy(gmergd[:, 2 * half + ch, :], gm_ps[:])
                gview = gmergd[:].rearrange("p a b -> p (a b)")
                gscr = pr.tile([128, 128], F32, tag="gdiag_scr")
                for k in range(C // 128):
                    nc.vector.scalar_tensor_tensor(
                        out=gscr[:], in0=gview[:, 128 * k : 128 * (k + 1)], scalar=0.0,
                        in1=ident[:], op0=OP.add, op1=OP.mult,
                        accum_out=gsl[:, k : k + 1],
                    )

            # weight loads, chunked 1MB so small DMAs interleave
            for c in range(8):
                nc.sync.dma_start(
                    wgt_sb[:, :, 512 * c : 512 * (c + 1)],
                    wgt_d[:, 512 * c : 512 * (c + 1)].rearrange("(o q) f -> q o f", q=128),
                )
            # =============== EXPERT FFN (bf16) ===============
            with (
                tc.tile_pool(name="ffn", bufs=1) as pf,
                tc.tile_pool(name="ffn_db", bufs=2) as pfd,
                tc.tile_pool(name="ffn_drow", bufs=4) as pfg,
                tc.tile_pool(name="psum_mm", bufs=2, space="PSUM") as psm,
            ):
                wdn_sb = pf.tile([128, F // 128, D], BF16)
                # 128-row strided view of rs_in for the scatters: cost-model
                # sized to what is actually written, yet overlapping every
                # zero-fill chunk so Tile orders zeros -> scatters -> RS.
                scat_view = rs_in.rearrange("(a b) d -> b a d", b=64)[0]
                eo_tiles = {}
                dispT_tiles = {}

                def dispatch(cb):
                    # gather 4 x 128 slot rows; XBAR DMA transpose into dispT
                    # (keeps dispatch off the in-order PE queue entirely):
                    # dispT[p, j, c] = drow[c, 128j + p]
                    dispT = pfd.tile([128, D // 128, CBLK], BF16, tag="dispT")
                    dispT_tiles[cb] = dispT
                    for kt in range(CBLK // 128):
                        k = (CBLK // 128) * cb + kt
                        drow = pfg.tile([128, D], BF16, tag="drow")
                        nc.gpsimd.indirect_dma_start(
                            out=drow[:], out_offset=None, in_=xb[:],
                            in_offset=bass.IndirectOffsetOnAxis(ap=tokci[:, k : k + 1], axis=0),
                        )
                        nc.sync.dma_start_transpose(
                            dispT[:, :, 128 * kt : 128 * (kt + 1)], drow[:]
                        )
                    return drow

                dispatch(0)
                drow_gate = dispatch(1)
                # bulk loads gated behind block-1's last gather via a REAL
                # data dep (gate cells computed from drow_gate): Tile
                # schedules by dependency, not program order, so only a true
                # RAW edge keeps these DMAs out of the DMA queue until the
                # critical-path dispatch is done.
                gate_b = drow_gate[:, 0:1].to_broadcast([128, 2, D])
                for c in range(8):
                    nc.vector.tensor_scalar(
                        out=wdn_sb[:, 4 * c : 4 * c + 1, 0:1],
                        in0=drow_gate[:, 0:1], scalar1=0.0, scalar2=None, op0=OP.mult,
                    )
                    nc.sync.dma_start(
                        wdn_sb[:, 4 * c : 4 * (c + 1), :],
                        wdn_d[512 * c : 512 * (c + 1), :].rearrange("(o q) d -> q o d", q=128),
                    )
                nc.vector.tensor_scalar(
                    out=zt[:], in0=gate_b, scalar1=0.0, scalar2=None, op0=OP.mult,
                )
                for c in range(32):
                    nc.sync.dma_start(
                        rs_in[256 * c : 256 * (c + 1), :].rearrange("(q p) d -> p q d", p=128),
                        zt[:],
                    )

                for cb in range(NCB):
                    if cb >= 2:
                        dispatch(cb)
                    dispT = dispT_tiles.pop(cb)
                    if cb > 0:
                        # return previous block's gated rows to rs_in[token]
                        eo_prev = eo_tiles.pop(cb - 1)
                        for ct in range(CBLK // 128):
                            kprev = (CBLK // 128) * (cb - 1) + ct
                            nc.gpsimd.indirect_dma_start(
                                out=scat_view, in_=eo_prev[:, ct, :], in_offset=None,
                                out_offset=bass.IndirectOffsetOnAxis(ap=tokci[:, kprev : kprev + 1], axis=0),
                                bounds_check=T - 1, oob_is_err=False,
                            )

                    hT = pf.tile([128, F // 128, CBLK], BF16, tag="hT")
                    for ft in range(F // 128):
                        ps1 = psm.tile([128, CBLK], F32, space="PSUM", tag="ps1")
                        for kd in range(D // 128):
                            nc.tensor.matmul(
                                ps1[:],
                                lhsT=wgt_sb[:, kd, 128 * ft : 128 * ft + 128],
                                rhs=dispT[:, kd, :],
                                start=(kd == 0), stop=(kd == D // 128 - 1),
                            )
                        nc.scalar.activation(hT[:, ft, :], ps1[:], AF.Gelu)
                    # mm2 with swapped operands: eo[c, d] = g * (hT.T @ w_down)
                    eo_sb = pfd.tile([128, CBLK // 128, D], BF16, tag="eo_sb")
                    eo_tiles[cb] = eo_sb
                    for ct in range(CBLK // 128):
                        kcur = (CBLK // 128) * cb + ct
                        for dc in range(D // 512):
                            ps2 = psm.tile([128, 512], F32, space="PSUM", tag="ps2")
                            for ft in range(F // 128):
                                nc.tensor.matmul(
                                    ps2[:],
                                    lhsT=hT[:, ft, 128 * ct : 128 * ct + 128],
                                    rhs=wdn_sb[:, ft, 512 * dc : 512 * dc + 512],
                                    start=(ft == 0), stop=(ft == F // 128 - 1),
                                )
                            nc.vector.tensor_scalar_mul(
                                eo_sb[:, ct, 512 * dc : 512 * dc + 512], ps2[:],
                                gsl[:, kcur : kcur + 1],
                            )

                # last block's return scatters
                eo_prev = eo_tiles.pop(NCB - 1)
                for ct in range(CBLK // 128):
                    kprev = (CBLK // 128) * (NCB - 1) + ct
                    nc.gpsimd.indirect_dma_start(
                        out=scat_view, in_=eo_prev[:, ct, :], in_offset=None,
                        out_offset=bass.IndirectOffsetOnAxis(ap=tokci[:, kprev : kprev + 1], axis=0),
                        bounds_check=T - 1, oob_is_err=False,
                    )

                # sum the two expert contributions per token; shard m -> core m
                nc.gpsimd.collective_compute(
                    "ReduceScatter", OP.add,
                    replica_groups=[list(range(NC))],
                    ins=[rs_in[:].opt()], outs=[rs_out[:].opt()],
                )
                nc.sync.dma_start(y_d[:], rs_out[:])

    nc.compile()
    return nc


_PROGRAM = None


def _get_program():
    global _PROGRAM
    if _PROGRAM is None:
        _PROGRAM = _build_program()
    return _PROGRAM


def host_constants():
    p = np.arange(128)
    return {
        "ident": np.eye(128, dtype=np.float32),
        "slmat": (np.arange(128)[None, :] > p[:, None]).astype(np.float32),
        "tidx": (64 * p[:, None] + np.arange(64)[None, :]).astype(np.float32),
        "eidx": np.tile(np.arange(E, dtype=np.float32), (128, 1)),
    }


def _make_in_maps(x, wg, w_gate, w_down):
    x = np.asarray(x, np.float32)
    wg_np = np.asarray(wg, np.float32)
    w_gate_np = np.asarray(w_gate, np.float32)
    w_down_np = np.asarray(w_down, np.float32)

    tokens = x.reshape(T, D)
    xb = np.zeros((T + 1, D), ml_dtypes.bfloat16)
    xb[:T] = tokens.astype(ml_dtypes.bfloat16)

    # shard m holds tokens [SH*m, SH*(m+1)); its xT columns are permuted so that
    # matmul tile position j = 128*tt + p corresponds to local token 8*p + tt,
    # making the routing payload DMA contiguous.
    j = np.arange(SH)
    perm = 8 * (j % 128) + j // 128  # local token index at column position j
    consts = host_constants()

    in_maps = []
    for m in range(NC):
        shard = tokens[SH * m : SH * (m + 1)]
        xT_sh = np.ascontiguousarray(shard[perm].T)
        in_maps.append({
            "xT_sh": xT_sh,
            "xb": xb,
            "wg": wg_np,
            "wgt": np.ascontiguousarray(w_gate_np[m].astype(ml_dtypes.bfloat16)),
            "wdn": np.ascontiguousarray(w_down_np[m].astype(ml_dtypes.bfloat16)),
            "cid": np.full((128, 1), float(m), np.float32),
            **consts,
        })
    return in_maps


def kernel(x, wg, w_gate, w_down, _trace=False):
    global LAST_RESULT
    x = np.asarray(x, np.float32)
    in_maps = _make_in_maps(x, wg, w_gate, w_down)
    nc = _get_program()
    res = run_bass_kernel_spmd(nc, in_maps, core_ids=list(range(NC)), trace=_trace)
    LAST_RESULT = res
    out = np.concatenate([res.results[m]["y"] for m in range(NC)], axis=0)
    return out.reshape(B, S, D).astype(x.dtype)


def bench(x, wg, w_gate, w_down, iters=6):
    """Measure per-execution wall time with device-resident inputs.

    Returns (output, per_call_seconds_list).
    """
    import time
    import jax
    from jax.sharding import Mesh, PartitionSpec, NamedSharding
    from jax.experimental.shard_map import shard_map
    import concourse.mybir as _mybir
    from concourse.bass2jax import _bass_exec_p, install_neuronx_cc_hook, partition_id_tensor

    install_neuronx_cc_hook()
    nc = _get_program()

    x = np.asarray(x, np.float32)
    in_maps = _make_in_maps(x, wg, w_gate, w_down)

    in_names, out_names, out_avals, zero_outs = [], [], [], []
    for alloc in nc.m.functions[0].allocations:
        if not isinstance(alloc, _mybir.MemoryLocationSet):
            continue
        name = alloc.memorylocations[0].name
        if alloc.kind == "ExternalInput":
            if nc.partition_id_tensor is None or name != nc.partition_id_tensor.name:
                in_names.append(name)
        elif alloc.kind == "ExternalOutput":
            shape = tuple(alloc.tensor_shape)
            dtype = _mybir.dt.np(alloc.dtype)
            out_names.append(name)
            out_avals.append(jax.core.ShapedArray(shape, dtype))
            zero_outs.append(np.zeros(shape, dtype))
    n_params = len(in_names)
    all_in_names = in_names + out_names
    if nc.partition_id_tensor is not None:
        all_in_names = all_in_names + [nc.partition_id_tensor.name]

    def _body(*args):
        operands = list(args)
        if nc.partition_id_tensor is not None:
            operands.append(partition_id_tensor())
        outs = _bass_exec_p.bind(
            *operands,
            out_avals=tuple(out_avals),
            in_names=tuple(all_in_names),
            out_names=tuple(out_names),
            lowering_input_output_aliases=(),
            sim_require_finite=True,
            sim_require_nnan=True,
            nc=nc,
        )
        return tuple(outs)

    devices = jax.devices()[:NC]
    mesh = Mesh(np.asarray(devices), ("core",))
    nsh = NamedSharding(mesh, PartitionSpec("core"))
    n_outs = len(out_avals)
    donate = tuple(range(n_params, n_params + n_outs))
    sharded = jax.jit(
        shard_map(_body, mesh=mesh, in_specs=(PartitionSpec("core"),) * (n_params + n_outs),
                  out_specs=(PartitionSpec("core"),) * n_outs, check_rep=False),
        donate_argnums=donate, keep_unused=True,
    )

    concat_in = [
        jax.device_put(np.concatenate([np.asarray(in_maps[c][nm]) for c in range(NC)], axis=0), nsh)
        for nm in in_names
    ]
    zero_sets = [
        [jax.device_put(np.zeros((NC * z.shape[0], *z.shape[1:]), z.dtype), nsh) for z in zero_outs]
        for _ in range(iters + 1)
    ]

    out = sharded(*concat_in, *zero_sets[0])  # warmup + compile
    jax.block_until_ready(out)
    times = []
    for it in range(iters):
        t0 = time.perf_counter()
        out = sharded(*concat_in, *zero_sets[it + 1])
        jax.block_until_ready(out)
        times.append(time.perf_counter() - t0)

    outs = {
        nm: np.asarray(out[i]).reshape(NC, *out_avals[i].shape) for i, nm in enumerate(out_names)
    }
    y = np.concatenate([outs["y"][m] for m in range(NC)], axis=0).reshape(B, S, D).astype(x.dtype)
    return y, times
